# revision 42
# baseline (speedup 1.0000x reference)
"""Trainium2 Bass kernel for DiamondLayer.

Computes out[b, d] = mean(x[b, d:d+16, d+17:d+33]) for d in [0, 2016):
16x16 mean-pool windows sliding along the diagonal of each 2048x2048 matrix.

Sharding: pure data parallel over batch - 32 batches -> 8 cores x 4 batches.

Per-core kernel (raw bacc, no Tile):
  - Only the diagonal band cols [r+2, r+34) of row r is ever touched, so each
    core DMAs just that band with a strided access pattern: partition p holds
    rows [16p, 16p+16), one 128B run per row (2016+16 descriptors/batch).
    One band DMA per batch on the SP ring (126 partitions - the HWDGE spray
    across 14 SDMA engines needs the outer AP count divisible by 14) plus a
    partition-126 tail DMA on the ACT ring.
  - VectorE computes a per-partition prefix scan P of the flat band
    (tensor_tensor_scan); window sums become differences of P.
  - The halo'd, prescaled prefix buffer PPH = P/256 is built in PSUM by two
    idle engines: ACT copies P[q, 0:512) to bank 0 (activation Copy with
    scale=1/256) and PE copies P[q+1, 0:465) to bank 1 via a matmul with a
    (1/256)*shift-by-one weight matrix (fed as an extra kernel input) -
    replacing the SBUF->SBUF halo DMA, which used to steal SDMA descriptor
    throughput from the band and cost ~3us of tail latency. Junk warmup
    matmuls release the PE HAM clock throttle first.
  - out[16q+u] = sum_s PPH[32u+31s+31] - sum_s PPH[32u+31s+15]: two strided
    DVE reduces per batch (1/256 already applied); the final subtract runs
    on GPSIMD for batches 0-2 (SBUF operands only - GPSIMD cannot touch
    PSUM) and on DVE for batch 3, shortening the DVE tail.
  - Two merged output DMAs on the SP ring (batches 0-2, then batch 3).
"""

import os
import sys

import numpy as np

for _p in ("/opt/trn_rl_repo",):
    if _p not in sys.path:
        sys.path.insert(0, _p)

B_FULL = 32
N_CORES = 8
B_PER_CORE = B_FULL // N_CORES  # 4
MAT = 2048
ND = MAT - 32  # 2016
NQ = ND // 16  # 126
NP = NQ + 1  # 127
ROW_STRIDE = MAT + 1  # 2049
MAT_ELEMS = MAT * MAT
BTW = 1024  # band buffer pitch (cols 0..512 used)
PPW = 544  # prefix buffer pitch (cols 0..512 used)
HALO = 465  # halo columns: max index 32*15+31*15+31 = 976 -> 976-512+1

LAST_EXEC_TIME_NS = None
_COMPILED = None


def _ensure_axon_ntff_hook():
    """This image's antenv lacks axon_hooks; bass_utils hard-imports it when
    trace=True under axon. Recreate the module and install the ctypes-based
    NTFF hook the boot shim would have installed. Degrades to no-op."""
    try:
        from antenv import axon_hooks  # noqa: F401

        return
    except ImportError:
        pass
    try:
        import types

        import antenv

        m = types.ModuleType("antenv.axon_hooks")
        _hook = [None]
        m.set_axon_ntff_profile_hook = lambda h: _hook.__setitem__(0, h)
        m.get_axon_ntff_profile_hook = lambda: _hook[0]
        sys.modules["antenv.axon_hooks"] = m
        antenv.axon_hooks = m
        if "/root/.axon_site" not in sys.path:
            sys.path.insert(0, "/root/.axon_site")
        from trn_agent_boot import trn_boot

        hook = trn_boot._ntff_profile_via_ctypes("/opt/axon/libaxon_pjrt.so")
        if hook is not None:
            m.set_axon_ntff_profile_hook(hook)
    except Exception:
        pass


def _make_weights() -> np.ndarray:
    """[127, 256] f32: cols 0..125 = (1/256)*I (PE copy of P[q]),
    cols 128..253 = (1/256)*shift (copy of P[q+1]; shift[p,oc]=1 iff p==oc+1)."""
    w = np.zeros((NP, 256), dtype=np.float32)
    q = np.arange(NQ)
    w[q, q] = 1.0 / 256.0
    w[q + 1, 128 + q] = 1.0 / 256.0
    return w


def _build():
    import concourse.bass as bass
    import concourse.bacc as bacc
    from concourse import mybir
    from contextlib import ExitStack

    f32 = mybir.dt.float32
    add = mybir.AluOpType.add
    sub_op = mybir.AluOpType.subtract
    bypass = mybir.AluOpType.bypass
    X = mybir.AxisListType.X

    nc = bacc.Bacc("TRN2", target_bir_lowering=False, debug=False)
    x = nc.dram_tensor("x", [B_PER_CORE, MAT, MAT], f32, kind="ExternalInput")
    w = nc.dram_tensor("w", [NP, 256], f32, kind="ExternalInput")
    y = nc.dram_tensor("y", [B_PER_CORE, ND], f32, kind="ExternalOutput")

    def v(t, off, pat):
        return bass.AP(t, off, pat)

    with ExitStack() as ctx:
        B = B_PER_CORE
        e = ctx.enter_context
        bts = [e(nc.sbuf_tensor(f"bt{i}", [NP, BTW], f32)) for i in range(B)]
        pps = [e(nc.sbuf_tensor(f"pp{i}", [NP, PPW], f32)) for i in range(B)]
        wt = e(nc.sbuf_tensor("wt", [NP, 256], f32))
        ro = e(nc.sbuf_tensor("ro", [NQ, 64], f32))
        jnk = e(nc.sbuf_tensor("jnk", [NP, 128], f32))
        rs1 = [e(nc.sbuf_tensor(f"r1_{i}", [NQ, 16], f32)) for i in range(B)]
        rs2 = [e(nc.sbuf_tensor(f"r2_{i}", [NQ, 16], f32)) for i in range(B)]
        pph = [nc.alloc_psum_tensor(f"ph{i}", [NQ, 1024], f32) for i in range(B)]
        bsem = [e(nc.semaphore(f"bsem{i}")) for i in range(B)]
        tsem = [e(nc.semaphore(f"tsem{i}")) for i in range(B)]
        wsem = e(nc.semaphore("wsem"))
        gsem = e(nc.semaphore("gsem"))
        vscan = e(nc.semaphore("vscan"))
        mmsem = e(nc.semaphore("mmsem"))
        acsem = e(nc.semaphore("acsem"))
        vred = e(nc.semaphore("vred"))
        psem = e(nc.semaphore("psem"))
        vec_done = e(nc.semaphore("vec_done"))
        dma_out = e(nc.semaphore("dma_out"))
        block = e(nc.Block(no_gpsimd_drain=True))

        @block.sync
        def _(sync):
            for b in range(B):
                # band: bt[p, 1+32t+j] = x[b, 16p+t, 16p+t+2+j], j in [0,32)
                sync.dma_start(
                    v(bts[b], 1, [[BTW, NQ], [32, 16], [1, 32]]),
                    bass.AP(
                        x,
                        b * MAT_ELEMS + 2,
                        [[16 * ROW_STRIDE, NQ], [ROW_STRIDE, 16], [1, 32]],
                    ),
                ).then_inc(bsem[b], 16)
            # merged outputs: y[b, 16q+u] <- ro[q, 16b+u]; batch 3 separate
            # so batches 0-2 overlap the tail
            sync.wait_ge(psem, B - 1)
            sync.dma_start(
                bass.AP(y, 0, [[16, NQ], [ND, B - 1], [1, 16]]),
                v(ro, 0, [[64, NQ], [16, B - 1], [1, 16]]),
            ).then_inc(dma_out, 16)
            sync.wait_ge(psem, B - 1)
            sync.wait_ge(vec_done, 1)
            sync.dma_start(
                bass.AP(y, (B - 1) * ND, [[16, NQ], [1, 16]]),
                v(ro, 16 * (B - 1), [[64, NQ], [1, 16]]),
            ).then_inc(dma_out, 16)
            sync.wait_ge(dma_out, 32)

        @block.scalar
        def _(scalar):
            for b in range(B):
                # partition 126's band rows (halo source for q=125)
                scalar.dma_start(
                    v(bts[b], NQ * BTW + 1, [[BTW, 1], [32, 16], [1, 32]]),
                    bass.AP(
                        x,
                        b * MAT_ELEMS + 2 + NQ * 16 * ROW_STRIDE,
                        [[16 * ROW_STRIDE, 1], [ROW_STRIDE, 16], [1, 32]],
                    ),
                ).then_inc(tsem[b], 16)
            # w-load after the tails: its 258KB stops competing with band 0's
            # early SDMA window; wsem still fires well before the first real
            # matmul needs it (warmups read jnk, not wt). 126+1 split keeps
            # the big piece's outer count spray-friendly.
            scalar.dma_start(
                v(wt, 0, [[256, NQ], [1, 256]]),
                bass.AP(w, 0, [[256, NQ], [1, 256]]),
            ).then_inc(wsem, 16)
            scalar.dma_start(
                v(wt, NQ * 256, [[256, 1], [1, 256]]),
                bass.AP(w, NQ * 256, [[256, 1], [1, 256]]),
            ).then_inc(wsem, 16)
            for b in range(B):
                # PPH[q, f] = P[q, f]/256 (PSUM bank 0) on the idle ACT engine
                scalar.wait_ge(vscan, b + 1)
                nc.scalar.activation(
                    out=v(pph[b], 0, [[1024, NQ], [1, 512]]),
                    in_=v(pps[b], 0, [[PPW, NQ], [1, 512]]),
                    func=mybir.ActivationFunctionType.Copy,
                    bias=0.0,
                    scale=1.0 / 256.0,
                ).then_inc(acsem, 1)

        @block.tensor
        def _(tensor):
            # PPH[q, 512+g] = P[q+1, g]/256 (bank 1): the PE's shift matmul
            # replaces the SBUF->SBUF halo DMA; junk warmup matmuls first to
            # release the HAM clock throttle before the real ones arrive.
            tensor.wait_ge(gsem, 1)
            for _ in range(6):
                nc.tensor.matmul(
                    v(pph[0], 512, [[1024, NQ], [1, 64]]),
                    v(jnk, 0, [[128, NP], [1, NQ]]),
                    v(jnk, 0, [[128, NP], [1, 64]]),
                    start=True,
                    stop=True,
                )
            tensor.wait_ge(wsem, 32)
            for b in range(B):
                tensor.wait_ge(vscan, b + 1)
                nc.tensor.matmul(
                    v(pph[b], 512, [[1024, NQ], [1, HALO]]),
                    v(wt, 128, [[256, NP], [1, NQ]]),
                    v(pps[b], 0, [[PPW, NP], [1, HALO]]),
                    start=True,
                    stop=True,
                ).then_inc(mmsem, 1)

        @block.gpsimd
        def _(gpsimd):
            for b in range(B - 1):
                # out[:, 16b..] = R1 - R2 for batches 0-2 on the idle Pool
                gpsimd.wait_ge(vred, 2 * (b + 1))
                nc.gpsimd.tensor_tensor(
                    out=v(ro, 16 * b, [[64, NQ], [1, 16]]),
                    in0=v(rs1[b], 0, [[16, NQ], [1, 16]]),
                    in1=v(rs2[b], 0, [[16, NQ], [1, 16]]),
                    op=sub_op,
                ).then_inc(psem, 1)

        def rblk(vector, b):
            # out[16q+u] = sum_s PPH[32u+31s+31] - sum_s PPH[32u+31s+15]
            vector.wait_ge(mmsem, b + 1)
            vector.wait_ge(acsem, b + 1)
            nc.vector.reduce_sum(
                out=v(rs1[b], 0, [[16, NQ], [1, 16]]),
                in_=v(pph[b], 31, [[1024, NQ], [32, 16], [31, 16]]),
                axis=X,
            ).then_inc(vred, 1)
            nc.vector.reduce_sum(
                out=v(rs2[b], 0, [[16, NQ], [1, 16]]),
                in_=v(pph[b], 15, [[1024, NQ], [32, 16], [31, 16]]),
                axis=X,
            ).then_inc(vred, 1)
            if b == B - 1:
                vector.wait_ge(vred, 2 * (b + 1))
                nc.vector.tensor_tensor(
                    out=v(ro, 16 * b, [[64, NQ], [1, 16]]),
                    in0=v(rs1[b], 0, [[16, NQ], [1, 16]]),
                    in1=v(rs2[b], 0, [[16, NQ], [1, 16]]),
                    op=sub_op,
                ).then_inc(vec_done, 1)

        @block.vector
        def _(vector):
            # junk operand for the PE warmup matmuls
            nc.vector.memset(v(jnk, 0, [[128, NP], [1, 128]]), 0.0).then_inc(
                gsem, 1
            )
            for b in range(B):
                # band col 0: never DMA'd; zero so the scan emits P[0] = 0
                nc.vector.memset(
                    v(bts[b], 0, [[BTW, NP], [1, 1]]), 0.0
                ).then_inc(gsem, 1)
            vector.wait_ge(gsem, B + 1)
            for b in range(B):
                vector.wait_ge(bsem[b], 16)
                vector.wait_ge(tsem[b], 16)
                # P[f] = prefix sum of the flat band per partition; P[0] = 0
                nc.vector.tensor_tensor_scan(
                    out=v(pps[b], 0, [[PPW, NP], [1, 513]]),
                    data0=v(bts[b], 0, [[BTW, NP], [1, 513]]),
                    data1=v(bts[b], 0, [[BTW, NP], [1, 513]]),
                    initial=0.0,
                    op0=add,
                    op1=bypass,
                ).then_inc(vscan, 1)
                if b == 3:
                    rblk(vector, 0)
            rblk(vector, 1)
            rblk(vector, 2)
            rblk(vector, 3)

    nc.compile()
    return nc


def _get_compiled():
    global _COMPILED
    if _COMPILED is None:
        _COMPILED = _build()
    return _COMPILED


def kernel(x: np.ndarray) -> np.ndarray:
    global LAST_EXEC_TIME_NS
    from concourse.bass_utils import run_bass_kernel_spmd

    x = np.ascontiguousarray(np.asarray(x), dtype=np.float32)
    assert x.shape == (B_FULL, MAT, MAT), x.shape

    nc = _get_compiled()
    wmat = _make_weights()
    in_maps = [
        {"x": x[i * B_PER_CORE : (i + 1) * B_PER_CORE], "w": wmat}
        for i in range(N_CORES)
    ]
    trace = bool(int(os.environ.get("KERNEL_TRACE", "0")))
    if trace:
        _ensure_axon_ntff_hook()
        # test-only: keep NTFF artifacts local instead of uploading
        from concourse import bass_utils as _bu

        _bu.upload_artifacts = lambda tmpdir: tmpdir
    res = run_bass_kernel_spmd(
        nc, in_maps, core_ids=list(range(N_CORES)), trace=trace
    )
    LAST_EXEC_TIME_NS = res.exec_time_ns
    out = np.concatenate([res.results[i]["y"] for i in range(N_CORES)], axis=0)
    return out.astype(np.float32)


# revision 43
# speedup vs baseline: 1.0082x; 1.0082x over previous
"""Trainium2 Bass kernel for DiamondLayer.

Computes out[b, d] = mean(x[b, d:d+16, d+17:d+33]) for d in [0, 2016):
16x16 mean-pool windows sliding along the diagonal of each 2048x2048 matrix.

Sharding: pure data parallel over batch - 32 batches -> 8 cores x 4 batches.

Per-core kernel (raw bacc, no Tile):
  - Only the diagonal band cols [r+2, r+34) of row r is ever touched, so each
    core DMAs just that band with a strided access pattern: partition p holds
    rows [16p, 16p+16), one 128B run per row (2016+16 descriptors/batch).
    One band DMA per batch on the SP ring (126 partitions - the HWDGE spray
    across 14 SDMA engines needs the outer AP count divisible by 14) plus a
    partition-126 tail DMA on the ACT ring.
  - VectorE computes a per-partition prefix scan P of the flat band
    (tensor_tensor_scan); window sums become differences of P.
  - The halo'd, prescaled prefix buffer PPH = P/256 is built in PSUM by two
    idle engines: ACT copies P[q, 0:512) to bank 0 (activation Copy with
    scale=1/256) and PE copies P[q+1, 0:465) to bank 1 via a matmul with a
    (1/256)*shift-by-one weight matrix (fed as an extra kernel input) -
    replacing the SBUF->SBUF halo DMA, which used to steal SDMA descriptor
    throughput from the band and cost ~3us of tail latency. Junk warmup
    matmuls release the PE HAM clock throttle first.
  - out[16q+u] = sum_s PPH[32u+31s+31] - sum_s PPH[32u+31s+15]: two strided
    DVE reduces + a DVE subtract per batch (1/256 already applied).
  - Two merged output DMAs on the SP ring (batches 0-2, then batch 3).
"""

import os
import sys

import numpy as np

for _p in ("/opt/trn_rl_repo",):
    if _p not in sys.path:
        sys.path.insert(0, _p)

B_FULL = 32
N_CORES = 8
B_PER_CORE = B_FULL // N_CORES  # 4
MAT = 2048
ND = MAT - 32  # 2016
NQ = ND // 16  # 126
NP = NQ + 1  # 127
ROW_STRIDE = MAT + 1  # 2049
MAT_ELEMS = MAT * MAT
BTW = 1024  # band buffer pitch (cols 0..512 used)
PPW = 544  # prefix buffer pitch (cols 0..512 used)
HALO = 465  # halo columns: max index 32*15+31*15+31 = 976 -> 976-512+1

LAST_EXEC_TIME_NS = None
_COMPILED = None


def _ensure_axon_ntff_hook():
    """This image's antenv lacks axon_hooks; bass_utils hard-imports it when
    trace=True under axon. Recreate the module and install the ctypes-based
    NTFF hook the boot shim would have installed. Degrades to no-op."""
    try:
        from antenv import axon_hooks  # noqa: F401

        return
    except ImportError:
        pass
    try:
        import types

        import antenv

        m = types.ModuleType("antenv.axon_hooks")
        _hook = [None]
        m.set_axon_ntff_profile_hook = lambda h: _hook.__setitem__(0, h)
        m.get_axon_ntff_profile_hook = lambda: _hook[0]
        sys.modules["antenv.axon_hooks"] = m
        antenv.axon_hooks = m
        if "/root/.axon_site" not in sys.path:
            sys.path.insert(0, "/root/.axon_site")
        from trn_agent_boot import trn_boot

        hook = trn_boot._ntff_profile_via_ctypes("/opt/axon/libaxon_pjrt.so")
        if hook is not None:
            m.set_axon_ntff_profile_hook(hook)
    except Exception:
        pass


def _make_weights() -> np.ndarray:
    """[127, 256] f32: cols 0..125 = (1/256)*I (PE copy of P[q]),
    cols 128..253 = (1/256)*shift (copy of P[q+1]; shift[p,oc]=1 iff p==oc+1)."""
    w = np.zeros((NP, 256), dtype=np.float32)
    q = np.arange(NQ)
    w[q, q] = 1.0 / 256.0
    w[q + 1, 128 + q] = 1.0 / 256.0
    return w


def _build():
    import concourse.bass as bass
    import concourse.bacc as bacc
    from concourse import mybir
    from contextlib import ExitStack

    f32 = mybir.dt.float32
    add = mybir.AluOpType.add
    sub_op = mybir.AluOpType.subtract
    bypass = mybir.AluOpType.bypass
    X = mybir.AxisListType.X

    nc = bacc.Bacc("TRN2", target_bir_lowering=False, debug=False)
    x = nc.dram_tensor("x", [B_PER_CORE, MAT, MAT], f32, kind="ExternalInput")
    w = nc.dram_tensor("w", [NP, 256], f32, kind="ExternalInput")
    y = nc.dram_tensor("y", [B_PER_CORE, ND], f32, kind="ExternalOutput")

    def v(t, off, pat):
        return bass.AP(t, off, pat)

    with ExitStack() as ctx:
        B = B_PER_CORE
        e = ctx.enter_context
        bts = [e(nc.sbuf_tensor(f"bt{i}", [NP, BTW], f32)) for i in range(B)]
        pps = [e(nc.sbuf_tensor(f"pp{i}", [NP, PPW], f32)) for i in range(B)]
        wt = e(nc.sbuf_tensor("wt", [NP, 256], f32))
        ro = e(nc.sbuf_tensor("ro", [NQ, 64], f32))
        rs1 = [e(nc.sbuf_tensor(f"r1_{i}", [NQ, 16], f32)) for i in range(B)]
        rs2 = [e(nc.sbuf_tensor(f"r2_{i}", [NQ, 16], f32)) for i in range(B)]
        pph = [nc.alloc_psum_tensor(f"ph{i}", [NQ, 1024], f32) for i in range(B)]
        bsem = [e(nc.semaphore(f"bsem{i}")) for i in range(B)]
        tsem = [e(nc.semaphore(f"tsem{i}")) for i in range(B)]
        wsem = e(nc.semaphore("wsem"))
        gsem = e(nc.semaphore("gsem"))
        vscan = e(nc.semaphore("vscan"))
        mmsem = e(nc.semaphore("mmsem"))
        acsem = e(nc.semaphore("acsem"))
        vred = e(nc.semaphore("vred"))
        psem = e(nc.semaphore("psem"))
        vec_done = e(nc.semaphore("vec_done"))
        dma_out = e(nc.semaphore("dma_out"))
        block = e(nc.Block(no_gpsimd_drain=True))

        @block.sync
        def _(sync):
            for b in range(B):
                # band: bt[p, 1+32t+j] = x[b, 16p+t, 16p+t+2+j], j in [0,32)
                sync.dma_start(
                    v(bts[b], 1, [[BTW, NQ], [32, 16], [1, 32]]),
                    bass.AP(
                        x,
                        b * MAT_ELEMS + 2,
                        [[16 * ROW_STRIDE, NQ], [ROW_STRIDE, 16], [1, 32]],
                    ),
                ).then_inc(bsem[b], 16)
            # merged outputs: y[b, 16q+u] <- ro[q, 16b+u]; batch 3 separate
            # so batches 0-2 overlap the tail
            sync.wait_ge(psem, B - 1)
            sync.dma_start(
                bass.AP(y, 0, [[16, NQ], [ND, B - 1], [1, 16]]),
                v(ro, 0, [[64, NQ], [16, B - 1], [1, 16]]),
            ).then_inc(dma_out, 16)
            sync.wait_ge(psem, B - 1)
            sync.wait_ge(vec_done, 1)
            sync.dma_start(
                bass.AP(y, (B - 1) * ND, [[16, NQ], [1, 16]]),
                v(ro, 16 * (B - 1), [[64, NQ], [1, 16]]),
            ).then_inc(dma_out, 16)
            sync.wait_ge(dma_out, 32)

        @block.scalar
        def _(scalar):
            # split 126+1 so the big piece's outer count stays spray-friendly
            scalar.dma_start(
                v(wt, 0, [[256, NQ], [1, 256]]),
                bass.AP(w, 0, [[256, NQ], [1, 256]]),
            ).then_inc(wsem, 16)
            scalar.dma_start(
                v(wt, NQ * 256, [[256, 1], [1, 256]]),
                bass.AP(w, NQ * 256, [[256, 1], [1, 256]]),
            ).then_inc(wsem, 16)
            for b in range(B):
                # partition 126's band rows (halo source for q=125)
                scalar.dma_start(
                    v(bts[b], NQ * BTW + 1, [[BTW, 1], [32, 16], [1, 32]]),
                    bass.AP(
                        x,
                        b * MAT_ELEMS + 2 + NQ * 16 * ROW_STRIDE,
                        [[16 * ROW_STRIDE, 1], [ROW_STRIDE, 16], [1, 32]],
                    ),
                ).then_inc(tsem[b], 16)
            for b in range(B):
                # PPH[q, f] = P[q, f]/256 (PSUM bank 0) on the idle ACT engine
                scalar.wait_ge(vscan, b + 1)
                nc.scalar.activation(
                    out=v(pph[b], 0, [[1024, NQ], [1, 512]]),
                    in_=v(pps[b], 0, [[PPW, NQ], [1, 512]]),
                    func=mybir.ActivationFunctionType.Copy,
                    bias=0.0,
                    scale=1.0 / 256.0,
                ).then_inc(acsem, 1)

        @block.tensor
        def _(tensor):
            # PPH[q, 512+g] = P[q+1, g]/256 (bank 1): the PE's shift matmul
            # replaces the SBUF->SBUF halo DMA; junk warmup matmuls first to
            # release the HAM clock throttle before the real ones arrive.
            tensor.wait_ge(wsem, 32)
            for _ in range(6):
                nc.tensor.matmul(
                    v(pph[0], 512, [[1024, NQ], [1, 64]]),
                    v(wt, 128, [[256, NP], [1, NQ]]),
                    v(wt, 0, [[256, NP], [1, 64]]),
                    start=True,
                    stop=True,
                )
            for b in range(B):
                tensor.wait_ge(vscan, b + 1)
                nc.tensor.matmul(
                    v(pph[b], 512, [[1024, NQ], [1, HALO]]),
                    v(wt, 128, [[256, NP], [1, NQ]]),
                    v(pps[b], 0, [[PPW, NP], [1, HALO]]),
                    start=True,
                    stop=True,
                ).then_inc(mmsem, 1)

        @block.gpsimd
        def _(gpsimd):
            for b in range(B - 1):
                # out[:, 16b..] = R1 - R2 for batches 0-2 on the idle Pool
                gpsimd.wait_ge(vred, 2 * (b + 1))
                nc.gpsimd.tensor_tensor(
                    out=v(ro, 16 * b, [[64, NQ], [1, 16]]),
                    in0=v(rs1[b], 0, [[16, NQ], [1, 16]]),
                    in1=v(rs2[b], 0, [[16, NQ], [1, 16]]),
                    op=sub_op,
                ).then_inc(psem, 1)

        def rblk(vector, b):
            # out[16q+u] = sum_s PPH[32u+31s+31] - sum_s PPH[32u+31s+15]
            vector.wait_ge(mmsem, b + 1)
            vector.wait_ge(acsem, b + 1)
            nc.vector.reduce_sum(
                out=v(rs1[b], 0, [[16, NQ], [1, 16]]),
                in_=v(pph[b], 31, [[1024, NQ], [32, 16], [31, 16]]),
                axis=X,
            ).then_inc(vred, 1)
            nc.vector.reduce_sum(
                out=v(rs2[b], 0, [[16, NQ], [1, 16]]),
                in_=v(pph[b], 15, [[1024, NQ], [32, 16], [31, 16]]),
                axis=X,
            ).then_inc(vred, 1)
            if b == B - 1:
                vector.wait_ge(vred, 2 * (b + 1))
                nc.vector.tensor_tensor(
                    out=v(ro, 16 * b, [[64, NQ], [1, 16]]),
                    in0=v(rs1[b], 0, [[16, NQ], [1, 16]]),
                    in1=v(rs2[b], 0, [[16, NQ], [1, 16]]),
                    op=sub_op,
                ).then_inc(vec_done, 1)

        @block.vector
        def _(vector):
            for b in range(B):
                # band col 0: never DMA'd; zero so the scan emits P[0] = 0
                nc.vector.memset(
                    v(bts[b], 0, [[BTW, NP], [1, 1]]), 0.0
                ).then_inc(gsem, 1)
            vector.wait_ge(gsem, B)
            for b in range(B):
                vector.wait_ge(bsem[b], 16)
                vector.wait_ge(tsem[b], 16)
                # P[f] = prefix sum of the flat band per partition; P[0] = 0
                nc.vector.tensor_tensor_scan(
                    out=v(pps[b], 0, [[PPW, NP], [1, 513]]),
                    data0=v(bts[b], 0, [[BTW, NP], [1, 513]]),
                    data1=v(bts[b], 0, [[BTW, NP], [1, 513]]),
                    initial=0.0,
                    op0=add,
                    op1=bypass,
                ).then_inc(vscan, 1)
                if b == 3:
                    rblk(vector, 0)
            rblk(vector, 1)
            rblk(vector, 2)
            rblk(vector, 3)

    nc.compile()
    return nc


def _get_compiled():
    global _COMPILED
    if _COMPILED is None:
        _COMPILED = _build()
    return _COMPILED


def kernel(x: np.ndarray) -> np.ndarray:
    global LAST_EXEC_TIME_NS
    from concourse.bass_utils import run_bass_kernel_spmd

    x = np.ascontiguousarray(np.asarray(x), dtype=np.float32)
    assert x.shape == (B_FULL, MAT, MAT), x.shape

    nc = _get_compiled()
    wmat = _make_weights()
    in_maps = [
        {"x": x[i * B_PER_CORE : (i + 1) * B_PER_CORE], "w": wmat}
        for i in range(N_CORES)
    ]
    trace = bool(int(os.environ.get("KERNEL_TRACE", "0")))
    if trace:
        _ensure_axon_ntff_hook()
        # test-only: keep NTFF artifacts local instead of uploading
        from concourse import bass_utils as _bu

        _bu.upload_artifacts = lambda tmpdir: tmpdir
    res = run_bass_kernel_spmd(
        nc, in_maps, core_ids=list(range(N_CORES)), trace=trace
    )
    LAST_EXEC_TIME_NS = res.exec_time_ns
    out = np.concatenate([res.results[i]["y"] for i in range(N_CORES)], axis=0)
    return out.astype(np.float32)
